# revision 1
# baseline (speedup 1.0000x reference)
"""SageGCN segment-mean + residual + relu kernel for 8 Trainium2 NeuronCores.

Strategy:
  - Host: bucket nodes by segment length L (= action+1), deal nodes of each
    bucket round-robin across the 8 cores (balances rows/core), and pack each
    core's neighbor rows contiguously in bucket order.  Within a bucket every
    segment has the same length, so the segment-sum of a tile of K_L = W_L*L
    rows is a matmul with a constant block-diagonal 0/1 mask [K_L, W_L].
  - fp32 neighbor features are split host-side into bf16 hi + lo halves
    (exact to ~2^-16 relative), so the matmul runs at bf16 speed and
    accumulates hi and lo into the same PSUM region.
  - Epilogue: out = relu((0.5/L) * psum + 0.5*src) via one fused
    scalar_tensor_tensor on DVE plus one Relu activation on ACT, batched over
    B subtiles (one PSUM bank) at a time.
  - Host: unpermute the per-core outputs back to node order.

All cores share one SPMD NEFF: bucket sizes are padded to identical values
across cores, so the instruction stream is core-independent; only the DRAM
contents differ.
"""

import math
import os
import sys

import numpy as np

try:
    import concourse.bass as bass  # noqa: F401
except ImportError:
    sys.path.insert(0, "/opt/trn_rl_repo")

import ml_dtypes

import concourse.mybir as mybir
import concourse.tile as tile
from concourse import bacc, bass_utils

BF16 = ml_dtypes.bfloat16
N_CORES = 8
D = 128
G_SUB = 16  # subtiles per data DMA
B_SUB = 4  # subtiles per PSUM bank / epilogue batch

LAST_EXEC_TIME_NS = None
_CACHE = {}


def _make_plan(seg_len):
    """Bucket structure shared by all cores. seg_len: np.int64 [N]."""
    plan = []
    lengths = np.unique(seg_len)
    counts = {int(L): int((seg_len == L).sum()) for L in lengths}
    for L in sorted(counts):
        cnt = counts[L]
        if cnt == 0:
            continue
        assert 1 <= L <= 128, f"segment length {L} unsupported"
        W = 128 // L
        K = W * L
        C = -(-cnt // N_CORES)  # max nodes per core
        Cp = -(-C // W) * W  # pad to whole subtiles
        n_t = Cp // W
        plan.append(dict(L=L, W=W, K=K, cnt=cnt, Cp=Cp, n_t=n_t))
    return plan


def _iter_batches(plan):
    """Yield (bucket, b0, Bb, xoff) in the exact order used on device."""
    xoff = 0
    for b in plan:
        for b0 in range(0, b["n_t"], B_SUB):
            Bb = min(B_SUB, b["n_t"] - b0)
            yield b, b0, Bb, xoff
            xoff += b["W"] * Bb * D
    return


def _plan_sizes(plan):
    R = sum(b["n_t"] * b["K"] for b in plan)
    X = 0
    for b, b0, Bb, xoff in _iter_batches(plan):
        X = xoff + b["W"] * Bb * D
    sumW = sum(b["W"] for b in plan)
    return R, X, sumW


def _build_masks(plan, sumW):
    masks = np.zeros((128, sumW), dtype=BF16)
    moff = 0
    for b in plan:
        L, W, K = b["L"], b["W"], b["K"]
        k = np.arange(K)
        masks[k, moff + k // L] = BF16(1.0)
        moff += W
    return masks


def _build_nc(plan):
    R, X, sumW = _plan_sizes(plan)
    nc = bacc.Bacc("TRN2", target_bir_lowering=False, debug=False,
                   num_devices=N_CORES)
    data = nc.dram_tensor("data", [R, 2 * D], mybir.dt.bfloat16,
                          kind="ExternalInput").ap()
    src = nc.dram_tensor("src", [X], mybir.dt.float32,
                         kind="ExternalInput").ap()
    masks = nc.dram_tensor("masks", [128, sumW], mybir.dt.bfloat16,
                           kind="ExternalInput").ap()
    out = nc.dram_tensor("out", [X], mybir.dt.float32,
                         kind="ExternalOutput").ap()

    Relu = mybir.ActivationFunctionType.Relu
    Alu = mybir.AluOpType

    with tile.TileContext(nc) as tc:
        with tc.tile_pool(name="const", bufs=1) as cpool, \
             tc.tile_pool(name="data", bufs=3) as dpool, \
             tc.tile_pool(name="srcp", bufs=3) as spool, \
             tc.tile_pool(name="ps", bufs=4, space="PSUM") as ppool, \
             tc.tile_pool(name="tmp", bufs=3) as tpool, \
             tc.tile_pool(name="outp", bufs=3) as opool:
            mtile = cpool.tile([128, sumW], mybir.dt.bfloat16)
            nc.sync.dma_start(out=mtile[:, :], in_=masks[:, :])

            row0 = 0
            moff = 0
            xoff = 0
            for b in plan:
                L, W, K, n_t = b["L"], b["W"], b["K"], b["n_t"]
                lhsT = mtile[:K, moff:moff + W]
                for g0 in range(0, n_t, G_SUB):
                    Gg = min(G_SUB, n_t - g0)
                    dtile = dpool.tile([128, G_SUB * 2 * D],
                                       mybir.dt.bfloat16, tag="data")
                    dsl = data[row0 + g0 * K: row0 + (g0 + Gg) * K, :]
                    in3 = dsl.rearrange("(j p) c -> p j c", p=K)
                    out3 = dtile[:K, :Gg * 2 * D].rearrange(
                        "p (j c) -> p j c", c=2 * D)
                    nc.sync.dma_start(out=out3, in_=in3)
                    for b0 in range(0, Gg, B_SUB):
                        Bb = min(B_SUB, Gg - b0)
                        F = Bb * D
                        ptile = ppool.tile([128, B_SUB * D],
                                           mybir.dt.float32, tag="ps")
                        for j in range(Bb):
                            c0 = (b0 + j) * 2 * D
                            po = ptile[:W, j * D:(j + 1) * D]
                            nc.tensor.matmul(
                                po, lhsT, dtile[:K, c0:c0 + D],
                                start=(j == 0), stop=False)
                            nc.tensor.matmul(
                                po, lhsT, dtile[:K, c0 + D:c0 + 2 * D],
                                start=False, stop=(j == Bb - 1))
                        stile = spool.tile([128, B_SUB * D],
                                           mybir.dt.float32, tag="srcp")
                        nc.sync.dma_start(
                            out=stile[:W, :F],
                            in_=src[xoff:xoff + W * F].rearrange(
                                "(p f) -> p f", p=W))
                        ttile = tpool.tile([128, B_SUB * D],
                                           mybir.dt.float32, tag="tmp")
                        nc.vector.scalar_tensor_tensor(
                            ttile[:W, :F], ptile[:W, :F], float(0.5 / L),
                            stile[:W, :F], Alu.mult, Alu.add)
                        otile = opool.tile([128, B_SUB * D],
                                           mybir.dt.float32, tag="outp")
                        nc.scalar.activation(otile[:W, :F], ttile[:W, :F],
                                             Relu)
                        nc.sync.dma_start(
                            out=out[xoff:xoff + W * F].rearrange(
                                "(p f) -> p f", p=W),
                            in_=otile[:W, :F])
                        xoff += W * F
                row0 += n_t * K
                moff += W
    nc.compile()
    return nc


def _core_nodes(plan, seg_len, core):
    """Padded node list (−1 = padding) per bucket for one core."""
    per_bucket = []
    for b in plan:
        nodes = np.flatnonzero(seg_len == b["L"])[core::N_CORES]
        pad = np.full(b["Cp"], -1, dtype=np.int64)
        pad[:len(nodes)] = nodes
        per_bucket.append(pad)
    return per_bucket


def _pack_core(plan, node_lists, starts, nbr, src_half):
    R, X, _ = _plan_sizes(plan)
    row_idx = np.empty(R, dtype=np.int64)
    src_flat = np.empty(X, dtype=np.float32)
    ro = 0
    for b, nodes in zip(plan, node_lists):
        L, W, Cp, n_t = b["L"], b["W"], b["Cp"], b["n_t"]
        st = np.where(nodes >= 0, starts[np.maximum(nodes, 0)], 0)
        idx = (st[:, None] + np.arange(L)[None, :]).reshape(-1)
        row_idx[ro:ro + Cp * L] = idx
        ro += Cp * L
    data = nbr[row_idx]  # [R, 128] f32
    hi = data.astype(BF16)
    lo = (data - hi.astype(np.float32)).astype(BF16)
    data_flat = np.concatenate([hi, lo], axis=1)  # [R, 256] bf16

    bi = {id(b): i for i, b in enumerate(plan)}
    arr3_cache = {}
    for b, b0, Bb, xoff in _iter_batches(plan):
        W, n_t = b["W"], b["n_t"]
        key = id(b)
        if key not in arr3_cache:
            nodes = node_lists[bi[key]]
            vals = src_half[np.maximum(nodes, 0)]  # [Cp, 128] f32
            arr3_cache[key] = vals.reshape(n_t, W, D)
        blk = arr3_cache[key][b0:b0 + Bb]  # [Bb, W, D]
        src_flat[xoff:xoff + W * Bb * D] = \
            blk.transpose(1, 0, 2).reshape(-1)
    return data_flat, src_flat


def _unpack_core(plan, node_lists, out_flat, out_full):
    bi = {id(b): i for i, b in enumerate(plan)}
    vals_cache = {}
    for b, b0, Bb, xoff in _iter_batches(plan):
        W = b["W"]
        key = id(b)
        if key not in vals_cache:
            vals_cache[key] = np.empty((b["Cp"], D), dtype=np.float32)
        blk = out_flat[xoff:xoff + W * Bb * D].reshape(W, Bb, D)
        vals_cache[key][b0 * W:(b0 + Bb) * W] = \
            blk.transpose(1, 0, 2).reshape(Bb * W, D)
    for b, nodes in zip(plan, node_lists):
        vals = vals_cache[id(b)]
        real = nodes >= 0
        out_full[nodes[real]] = vals[real]


def kernel(action, src_node_features, neighbor_node_features):
    global LAST_EXEC_TIME_NS
    action = np.asarray(action).astype(np.int64)
    src = np.asarray(src_node_features, dtype=np.float32)
    nbr = np.asarray(neighbor_node_features, dtype=np.float32)
    N = action.shape[0]

    seg_len = action + 1
    starts = np.zeros(N, dtype=np.int64)
    np.cumsum(seg_len[:-1], out=starts[1:])

    plan = _make_plan(seg_len)
    key = tuple((b["L"], b["Cp"]) for b in plan)
    if key in _CACHE:
        nc = _CACHE[key]
    else:
        nc = _build_nc(plan)
        _CACHE[key] = nc

    _, _, sumW = _plan_sizes(plan)
    masks = _build_masks(plan, sumW)
    src_half = 0.5 * src

    in_maps = []
    node_lists_all = []
    for c in range(N_CORES):
        node_lists = _core_nodes(plan, seg_len, c)
        node_lists_all.append(node_lists)
        data_flat, src_flat = _pack_core(plan, node_lists, starts, nbr,
                                         src_half)
        in_maps.append({"data": data_flat, "src": src_flat, "masks": masks})

    trace = bool(int(os.environ.get("KERNEL_TRACE", "0")))
    res = bass_utils.run_bass_kernel_spmd(
        nc, in_maps, core_ids=list(range(N_CORES)), trace=trace)
    LAST_EXEC_TIME_NS = res.exec_time_ns

    out_full = np.empty((N, D), dtype=np.float32)
    for c in range(N_CORES):
        _unpack_core(plan, node_lists_all[c], res.results[c]["out"],
                     out_full)
    return out_full


# revision 2
# speedup vs baseline: 1.6571x; 1.6571x over previous
"""SageGCN segment-mean + residual + relu kernel for 8 Trainium2 NeuronCores.

Strategy:
  - Host: bucket nodes by segment length L (= action+1), deal nodes of each
    bucket round-robin across the 8 cores (balances rows/core), and pack each
    core's neighbor rows in bucket order.  Within a bucket every segment has
    the same length, so the segment-sum of a tile of K_L = W_L*L rows is a
    matmul with a constant block-diagonal 0/1 mask [K_L, W_L].
  - fp32 neighbor features are split host-side into bf16 hi + lo halves
    (exact to ~2^-16 relative), so the matmul runs at bf16 speed; hi and lo
    accumulate into the same PSUM region.  One matmul handles 4 subtiles
    (N=512) via a strided rhs access pattern.
  - Epilogue: out = relu((0.5/L) * psum + 0.5*src) via one fused
    scalar_tensor_tensor on DVE per PSUM bank plus one Relu activation on
    ACT per 16-subtile group.
  - Rows for each 16-subtile DMA group are packed partition-major host-side
    so every DMA moves long contiguous per-partition chunks.
  - Host: unpermute the per-core outputs back to node order.

All cores share one SPMD NEFF: bucket sizes are padded to identical values
across cores, so the instruction stream is core-independent; only the DRAM
contents differ.
"""

import math
import os
import sys

import numpy as np

try:
    import concourse.bass as bass  # noqa: F401
except ImportError:
    sys.path.insert(0, "/opt/trn_rl_repo")

import ml_dtypes

import concourse.mybir as mybir
import concourse.tile as tile
from concourse import bacc, bass_utils

BF16 = ml_dtypes.bfloat16
N_CORES = 8
D = 128
G_SUB = int(os.environ.get("K_GSUB", "16"))  # subtiles per data DMA group
B_SUB = 4  # subtiles per PSUM bank / matmul pair

LAST_EXEC_TIME_NS = None
_CACHE = {}


def _make_plan(seg_len):
    """Bucket structure shared by all cores. seg_len: np.int64 [N]."""
    plan = []
    lengths = np.unique(seg_len)
    counts = {int(L): int((seg_len == L).sum()) for L in lengths}
    for L in sorted(counts):
        cnt = counts[L]
        if cnt == 0:
            continue
        assert 1 <= L <= 128, f"segment length {L} unsupported"
        W = 128 // L
        K = W * L
        C = -(-cnt // N_CORES)  # max nodes per core
        Cp = -(-C // W) * W  # pad to whole subtiles
        n_t = Cp // W
        plan.append(dict(L=L, W=W, K=K, cnt=cnt, Cp=Cp, n_t=n_t))
    return plan


def _iter_groups(plan):
    """Yield (bucket, g0, Gg, rowbase, xoff) in device order.

    rowbase: starting row in the packed data array (p-major within group).
    xoff: starting element in the src/out flat arrays ([W, Gg*D] per group).
    """
    rowbase = 0
    xoff = 0
    for b in plan:
        for g0 in range(0, b["n_t"], G_SUB):
            Gg = min(G_SUB, b["n_t"] - g0)
            yield b, g0, Gg, rowbase, xoff
            rowbase += b["K"] * Gg
            xoff += b["W"] * Gg * D


def _plan_sizes(plan):
    R = 0
    X = 0
    for b, g0, Gg, rowbase, xoff in _iter_groups(plan):
        R = rowbase + b["K"] * Gg
        X = xoff + b["W"] * Gg * D
    sumW = sum(b["W"] for b in plan)
    return R, X, sumW


def _build_masks(plan, sumW):
    masks = np.zeros((128, sumW), dtype=BF16)
    moff = 0
    for b in plan:
        L, W, K = b["L"], b["W"], b["K"]
        k = np.arange(K)
        masks[k, moff + k // L] = BF16(1.0)
        moff += W
    return masks


def _build_nc(plan):
    R, X, sumW = _plan_sizes(plan)
    nc = bacc.Bacc("TRN2", target_bir_lowering=False, debug=False,
                   num_devices=N_CORES)
    data = nc.dram_tensor("data", [R, 2 * D], mybir.dt.bfloat16,
                          kind="ExternalInput").ap()
    src = nc.dram_tensor("src", [X], mybir.dt.float32,
                         kind="ExternalInput").ap()
    masks = nc.dram_tensor("masks", [128, sumW], mybir.dt.bfloat16,
                           kind="ExternalInput").ap()
    out = nc.dram_tensor("out", [X], mybir.dt.float32,
                         kind="ExternalOutput").ap()

    Relu = mybir.ActivationFunctionType.Relu
    Alu = mybir.AluOpType

    with tile.TileContext(nc) as tc:
        with tc.tile_pool(name="const", bufs=1) as cpool, \
             tc.tile_pool(name="data", bufs=3) as dpool, \
             tc.tile_pool(name="srcp", bufs=2) as spool, \
             tc.tile_pool(name="ps", bufs=4, space="PSUM") as ppool, \
             tc.tile_pool(name="tmp", bufs=2) as tpool, \
             tc.tile_pool(name="outp", bufs=2) as opool:
            mtile = cpool.tile([128, sumW], mybir.dt.bfloat16)
            nc.sync.dma_start(out=mtile[:, :], in_=masks[:, :])

            moffs = {}
            moff = 0
            for b in plan:
                moffs[b["L"]] = moff
                moff += b["W"]

            for b, g0, Gg, rowbase, xoff in _iter_groups(plan):
                L, W, K = b["L"], b["W"], b["K"]
                lhsT = mtile[:K, moffs[L]:moffs[L] + W]
                FG = Gg * D  # group free size for src/out

                dtile = dpool.tile([128, G_SUB * 2 * D],
                                   mybir.dt.bfloat16, tag="data")
                dsl = data[rowbase:rowbase + K * Gg, :]
                nc.sync.dma_start(
                    out=dtile[:K, :Gg * 2 * D],
                    in_=dsl.rearrange("(p j) c -> p (j c)", p=K))

                stile = spool.tile([128, G_SUB * D], mybir.dt.float32,
                                   tag="srcp")
                nc.gpsimd.dma_start(
                    out=stile[:W, :FG],
                    in_=src[xoff:xoff + W * FG].rearrange(
                        "(p f) -> p f", p=W))

                ttile = tpool.tile([128, G_SUB * D], mybir.dt.float32,
                                   tag="tmp")
                for b0 in range(0, Gg, B_SUB):
                    Bb = min(B_SUB, Gg - b0)
                    F = Bb * D
                    ptile = ppool.tile([128, B_SUB * D],
                                       mybir.dt.float32, tag="ps")
                    r3 = dtile[:K, b0 * 2 * D:(b0 + Bb) * 2 * D].rearrange(
                        "p (j c) -> p j c", c=2 * D)
                    nc.tensor.matmul(ptile[:W, :F], lhsT, r3[:, :, 0:D],
                                     start=True, stop=False)
                    nc.tensor.matmul(ptile[:W, :F], lhsT, r3[:, :, D:2 * D],
                                     start=False, stop=True)
                    nc.vector.scalar_tensor_tensor(
                        ttile[:W, b0 * D:b0 * D + F], ptile[:W, :F],
                        float(0.5 / L), stile[:W, b0 * D:b0 * D + F],
                        Alu.mult, Alu.add)

                otile = opool.tile([128, G_SUB * D], mybir.dt.float32,
                                   tag="outp")
                nc.scalar.activation(otile[:W, :FG], ttile[:W, :FG], Relu)
                nc.gpsimd.dma_start(
                    out=out[xoff:xoff + W * FG].rearrange(
                        "(p f) -> p f", p=W),
                    in_=otile[:W, :FG])
    nc.compile()
    return nc


def _core_nodes(plan, seg_len, core):
    """Padded node list (−1 = padding) per bucket for one core."""
    per_bucket = []
    for b in plan:
        nodes = np.flatnonzero(seg_len == b["L"])[core::N_CORES]
        pad = np.full(b["Cp"], -1, dtype=np.int64)
        pad[:len(nodes)] = nodes
        per_bucket.append(pad)
    return per_bucket


def _pack_core(plan, node_lists, starts, nbr, src_half):
    R, X, _ = _plan_sizes(plan)
    row_idx = np.empty(R, dtype=np.int64)
    src_flat = np.empty(X, dtype=np.float32)

    bi = {id(b): i for i, b in enumerate(plan)}
    rows_cache = {}
    src_cache = {}
    for b, g0, Gg, rowbase, xoff in _iter_groups(plan):
        L, W, K, n_t = b["L"], b["W"], b["K"], b["n_t"]
        key = id(b)
        if key not in rows_cache:
            nodes = node_lists[bi[key]]
            st = np.where(nodes >= 0, starts[np.maximum(nodes, 0)], 0)
            # row ids per subtile: [n_t, K]
            rows_cache[key] = (
                st[:, None] + np.arange(L)[None, :]
            ).reshape(n_t, K)
            src_cache[key] = src_half[np.maximum(nodes, 0)].reshape(
                n_t, W, D)
        # p-major within group: [K, Gg]
        blk = rows_cache[key][g0:g0 + Gg]  # [Gg, K]
        row_idx[rowbase:rowbase + K * Gg] = blk.T.reshape(-1)
        sblk = src_cache[key][g0:g0 + Gg]  # [Gg, W, D]
        src_flat[xoff:xoff + W * Gg * D] = \
            sblk.transpose(1, 0, 2).reshape(-1)

    data = nbr[row_idx]  # [R, 128] f32
    hi = data.astype(BF16)
    lo = (data - hi.astype(np.float32)).astype(BF16)
    data_flat = np.concatenate([hi, lo], axis=1)  # [R, 256] bf16
    return data_flat, src_flat


def _unpack_core(plan, node_lists, out_flat, out_full):
    bi = {id(b): i for i, b in enumerate(plan)}
    vals_cache = {}
    for b, g0, Gg, rowbase, xoff in _iter_groups(plan):
        W = b["W"]
        key = id(b)
        if key not in vals_cache:
            vals_cache[key] = np.empty((b["Cp"], D), dtype=np.float32)
        blk = out_flat[xoff:xoff + W * Gg * D].reshape(W, Gg, D)
        vals_cache[key][g0 * W:(g0 + Gg) * W] = \
            blk.transpose(1, 0, 2).reshape(Gg * W, D)
    for b, nodes in zip(plan, node_lists):
        vals = vals_cache[id(b)]
        real = nodes >= 0
        out_full[nodes[real]] = vals[real]


def kernel(action, src_node_features, neighbor_node_features):
    global LAST_EXEC_TIME_NS
    action = np.asarray(action).astype(np.int64)
    src = np.asarray(src_node_features, dtype=np.float32)
    nbr = np.asarray(neighbor_node_features, dtype=np.float32)
    N = action.shape[0]

    seg_len = action + 1
    starts = np.zeros(N, dtype=np.int64)
    np.cumsum(seg_len[:-1], out=starts[1:])

    plan = _make_plan(seg_len)
    key = tuple((b["L"], b["Cp"]) for b in plan)
    if key in _CACHE:
        nc = _CACHE[key]
    else:
        nc = _build_nc(plan)
        _CACHE[key] = nc

    _, _, sumW = _plan_sizes(plan)
    masks = _build_masks(plan, sumW)
    src_half = 0.5 * src

    in_maps = []
    node_lists_all = []
    for c in range(N_CORES):
        node_lists = _core_nodes(plan, seg_len, c)
        node_lists_all.append(node_lists)
        data_flat, src_flat = _pack_core(plan, node_lists, starts, nbr,
                                         src_half)
        in_maps.append({"data": data_flat, "src": src_flat, "masks": masks})

    trace = bool(int(os.environ.get("KERNEL_TRACE", "0")))
    res = bass_utils.run_bass_kernel_spmd(
        nc, in_maps, core_ids=list(range(N_CORES)), trace=trace)
    LAST_EXEC_TIME_NS = res.exec_time_ns

    out_full = np.empty((N, D), dtype=np.float32)
    for c in range(N_CORES):
        _unpack_core(plan, node_lists_all[c], res.results[c]["out"],
                     out_full)
    return out_full


# revision 3
# speedup vs baseline: 1.7179x; 1.0367x over previous
"""SageGCN segment-mean + residual + relu kernel for 8 Trainium2 NeuronCores.

Strategy:
  - Host: bucket nodes by segment length L (= action+1), deal nodes of each
    bucket round-robin across the 8 cores (balances rows/core), and pack each
    core's neighbor rows in bucket order.  Within a bucket every segment has
    the same length, so the segment-sum of a tile of K_L = W_L*L rows is a
    matmul with a constant block-diagonal 0/1 mask [K_L, W_L].
  - fp32 neighbor features are split host-side into bf16 hi + lo halves
    (exact to ~2^-16 relative), so the matmul runs at bf16 speed; hi and lo
    accumulate into the same PSUM region.  One matmul handles 4 subtiles
    (N=512) via a strided rhs access pattern.
  - Epilogue: out = relu((0.5/L) * psum + 0.5*src) via one fused
    scalar_tensor_tensor on DVE per PSUM bank plus one Relu activation on
    ACT per 16-subtile group.
  - Rows for each 16-subtile DMA group are packed partition-major host-side
    so every DMA moves long contiguous per-partition chunks.
  - Host: unpermute the per-core outputs back to node order.

All cores share one SPMD NEFF: bucket sizes are padded to identical values
across cores, so the instruction stream is core-independent; only the DRAM
contents differ.
"""

import math
import os
import sys

import numpy as np

try:
    import concourse.bass as bass  # noqa: F401
except ImportError:
    sys.path.insert(0, "/opt/trn_rl_repo")

import ml_dtypes

import concourse.mybir as mybir
import concourse.tile as tile
from concourse import bacc, bass_utils

BF16 = ml_dtypes.bfloat16
N_CORES = 8
D = 128
G_SUB = int(os.environ.get("K_GSUB", "16"))  # subtiles per data DMA group
B_SUB = 4  # subtiles per PSUM bank / matmul pair

LAST_EXEC_TIME_NS = None
_CACHE = {}


def _make_plan(seg_len):
    """Bucket structure shared by all cores. seg_len: np.int64 [N]."""
    plan = []
    lengths = np.unique(seg_len)
    counts = {int(L): int((seg_len == L).sum()) for L in lengths}
    for L in sorted(counts):
        cnt = counts[L]
        if cnt == 0:
            continue
        assert 1 <= L <= 128, f"segment length {L} unsupported"
        W = 128 // L
        K = W * L
        C = -(-cnt // N_CORES)  # max nodes per core
        Cp = -(-C // W) * W  # pad to whole subtiles
        n_t = Cp // W
        plan.append(dict(L=L, W=W, K=K, cnt=cnt, Cp=Cp, n_t=n_t))
    return plan


def _iter_groups(plan):
    """Yield (bucket, g0, Gg, rowbase, xoff) in device order.

    rowbase: starting row in the packed data array (p-major within group).
    xoff: starting element in the src/out flat arrays ([W, Gg*D] per group).
    """
    rowbase = 0
    xoff = 0
    for b in plan:
        for g0 in range(0, b["n_t"], G_SUB):
            Gg = min(G_SUB, b["n_t"] - g0)
            yield b, g0, Gg, rowbase, xoff
            rowbase += b["K"] * Gg
            xoff += b["W"] * Gg * D


def _plan_sizes(plan):
    R = 0
    X = 0
    for b, g0, Gg, rowbase, xoff in _iter_groups(plan):
        R = rowbase + b["K"] * Gg
        X = xoff + b["W"] * Gg * D
    sumW = sum(b["W"] for b in plan)
    return R, X, sumW


def _build_masks(plan, sumW):
    masks = np.zeros((128, sumW), dtype=BF16)
    moff = 0
    for b in plan:
        L, W, K = b["L"], b["W"], b["K"]
        k = np.arange(K)
        masks[k, moff + k // L] = BF16(1.0)
        moff += W
    return masks


def _build_nc(plan):
    R, X, sumW = _plan_sizes(plan)
    nc = bacc.Bacc("TRN2", target_bir_lowering=False, debug=False,
                   num_devices=N_CORES)
    data = nc.dram_tensor("data", [R, 2 * D], mybir.dt.bfloat16,
                          kind="ExternalInput").ap()
    src = nc.dram_tensor("src", [X], mybir.dt.float32,
                         kind="ExternalInput").ap()
    masks = nc.dram_tensor("masks", [128, sumW], mybir.dt.bfloat16,
                           kind="ExternalInput").ap()
    out = nc.dram_tensor("out", [X], mybir.dt.float32,
                         kind="ExternalOutput").ap()

    Relu = mybir.ActivationFunctionType.Relu
    Alu = mybir.AluOpType

    with tile.TileContext(nc) as tc:
        with tc.tile_pool(name="const", bufs=1) as cpool, \
             tc.tile_pool(name="data", bufs=4) as dpool, \
             tc.tile_pool(name="srcp", bufs=3) as spool, \
             tc.tile_pool(name="ps", bufs=6, space="PSUM") as ppool, \
             tc.tile_pool(name="tmp", bufs=2) as tpool, \
             tc.tile_pool(name="outp", bufs=3) as opool:
            mtile = cpool.tile([128, sumW], mybir.dt.bfloat16)
            nc.sync.dma_start(out=mtile[:, :], in_=masks[:, :])

            moffs = {}
            moff = 0
            for b in plan:
                moffs[b["L"]] = moff
                moff += b["W"]

            gi = 0
            for b, g0, Gg, rowbase, xoff in _iter_groups(plan):
                L, W, K = b["L"], b["W"], b["K"]
                lhsT = mtile[:K, moffs[L]:moffs[L] + W]
                FG = Gg * D  # group free size for src/out

                # Rotate the partition strip used for psum/src/out so the
                # partition-poor (small W) transfers spread across SDMA
                # engines. Offset must satisfy matmul col tile_position
                # alignment (32-aligned, within the rounded col size).
                if W <= 32:
                    o = 32 * (gi % 4)
                elif W <= 64:
                    o = 64 * (gi % 2)
                else:
                    o = 0
                gi += 1

                dtile = dpool.tile([128, G_SUB * 2 * D],
                                   mybir.dt.bfloat16, tag="data")
                dsl = data[rowbase:rowbase + K * Gg, :]
                nc.sync.dma_start(
                    out=dtile[:K, :Gg * 2 * D],
                    in_=dsl.rearrange("(p j) c -> p (j c)", p=K))

                stile = spool.tile([128, G_SUB * D], mybir.dt.float32,
                                   tag="srcp")
                nc.gpsimd.dma_start(
                    out=stile[o:o + W, :FG],
                    in_=src[xoff:xoff + W * FG].rearrange(
                        "(p f) -> p f", p=W))

                ttile = tpool.tile([128, G_SUB * D], mybir.dt.float32,
                                   tag="tmp")
                for b0 in range(0, Gg, B_SUB):
                    Bb = min(B_SUB, Gg - b0)
                    F = Bb * D
                    ptile = ppool.tile([128, B_SUB * D],
                                       mybir.dt.float32, tag="ps")
                    r3 = dtile[:K, b0 * 2 * D:(b0 + Bb) * 2 * D].rearrange(
                        "p (j c) -> p j c", c=2 * D)
                    nc.tensor.matmul(ptile[o:o + W, :F], lhsT,
                                     r3[:, :, 0:D],
                                     start=True, stop=False,
                                     tile_position=(0, o))
                    nc.tensor.matmul(ptile[o:o + W, :F], lhsT,
                                     r3[:, :, D:2 * D],
                                     start=False, stop=True,
                                     tile_position=(0, o))
                    nc.vector.scalar_tensor_tensor(
                        ttile[o:o + W, b0 * D:b0 * D + F],
                        ptile[o:o + W, :F],
                        float(0.5 / L), stile[o:o + W, b0 * D:b0 * D + F],
                        Alu.mult, Alu.add)

                otile = opool.tile([128, G_SUB * D], mybir.dt.float32,
                                   tag="outp")
                nc.scalar.activation(otile[o:o + W, :FG],
                                     ttile[o:o + W, :FG], Relu)
                nc.gpsimd.dma_start(
                    out=out[xoff:xoff + W * FG].rearrange(
                        "(p f) -> p f", p=W),
                    in_=otile[o:o + W, :FG])
    nc.compile()
    return nc


def _core_nodes(plan, seg_len, core):
    """Padded node list (−1 = padding) per bucket for one core."""
    per_bucket = []
    for b in plan:
        nodes = np.flatnonzero(seg_len == b["L"])[core::N_CORES]
        pad = np.full(b["Cp"], -1, dtype=np.int64)
        pad[:len(nodes)] = nodes
        per_bucket.append(pad)
    return per_bucket


def _pack_core(plan, node_lists, starts, nbr, src_half):
    R, X, _ = _plan_sizes(plan)
    row_idx = np.empty(R, dtype=np.int64)
    src_flat = np.empty(X, dtype=np.float32)

    bi = {id(b): i for i, b in enumerate(plan)}
    rows_cache = {}
    src_cache = {}
    for b, g0, Gg, rowbase, xoff in _iter_groups(plan):
        L, W, K, n_t = b["L"], b["W"], b["K"], b["n_t"]
        key = id(b)
        if key not in rows_cache:
            nodes = node_lists[bi[key]]
            st = np.where(nodes >= 0, starts[np.maximum(nodes, 0)], 0)
            # row ids per subtile: [n_t, K]
            rows_cache[key] = (
                st[:, None] + np.arange(L)[None, :]
            ).reshape(n_t, K)
            src_cache[key] = src_half[np.maximum(nodes, 0)].reshape(
                n_t, W, D)
        # p-major within group: [K, Gg]
        blk = rows_cache[key][g0:g0 + Gg]  # [Gg, K]
        row_idx[rowbase:rowbase + K * Gg] = blk.T.reshape(-1)
        sblk = src_cache[key][g0:g0 + Gg]  # [Gg, W, D]
        src_flat[xoff:xoff + W * Gg * D] = \
            sblk.transpose(1, 0, 2).reshape(-1)

    data = nbr[row_idx]  # [R, 128] f32
    hi = data.astype(BF16)
    lo = (data - hi.astype(np.float32)).astype(BF16)
    data_flat = np.concatenate([hi, lo], axis=1)  # [R, 256] bf16
    return data_flat, src_flat


def _unpack_core(plan, node_lists, out_flat, out_full):
    bi = {id(b): i for i, b in enumerate(plan)}
    vals_cache = {}
    for b, g0, Gg, rowbase, xoff in _iter_groups(plan):
        W = b["W"]
        key = id(b)
        if key not in vals_cache:
            vals_cache[key] = np.empty((b["Cp"], D), dtype=np.float32)
        blk = out_flat[xoff:xoff + W * Gg * D].reshape(W, Gg, D)
        vals_cache[key][g0 * W:(g0 + Gg) * W] = \
            blk.transpose(1, 0, 2).reshape(Gg * W, D)
    for b, nodes in zip(plan, node_lists):
        vals = vals_cache[id(b)]
        real = nodes >= 0
        out_full[nodes[real]] = vals[real]


def kernel(action, src_node_features, neighbor_node_features):
    global LAST_EXEC_TIME_NS
    action = np.asarray(action).astype(np.int64)
    src = np.asarray(src_node_features, dtype=np.float32)
    nbr = np.asarray(neighbor_node_features, dtype=np.float32)
    N = action.shape[0]

    seg_len = action + 1
    starts = np.zeros(N, dtype=np.int64)
    np.cumsum(seg_len[:-1], out=starts[1:])

    plan = _make_plan(seg_len)
    key = tuple((b["L"], b["Cp"]) for b in plan)
    if key in _CACHE:
        nc = _CACHE[key]
    else:
        nc = _build_nc(plan)
        _CACHE[key] = nc

    _, _, sumW = _plan_sizes(plan)
    masks = _build_masks(plan, sumW)
    src_half = 0.5 * src

    in_maps = []
    node_lists_all = []
    for c in range(N_CORES):
        node_lists = _core_nodes(plan, seg_len, c)
        node_lists_all.append(node_lists)
        data_flat, src_flat = _pack_core(plan, node_lists, starts, nbr,
                                         src_half)
        in_maps.append({"data": data_flat, "src": src_flat, "masks": masks})

    trace = bool(int(os.environ.get("KERNEL_TRACE", "0")))
    res = bass_utils.run_bass_kernel_spmd(
        nc, in_maps, core_ids=list(range(N_CORES)), trace=trace)
    LAST_EXEC_TIME_NS = res.exec_time_ns

    out_full = np.empty((N, D), dtype=np.float32)
    for c in range(N_CORES):
        _unpack_core(plan, node_lists_all[c], res.results[c]["out"],
                     out_full)
    return out_full


# revision 4
# speedup vs baseline: 5.6444x; 3.2857x over previous
"""SageGCN segment-mean + residual + relu kernel for 8 Trainium2 NeuronCores.

Strategy:
  - Host: bucket nodes by segment length L (= action+1), deal nodes of each
    bucket round-robin across the 8 cores (balances rows/core), and pack each
    core's rows in bucket order.  Each segment is stored as its L neighbor
    rows followed by the node's own src row.  Within a bucket every segment
    spans S = L+1 rows, so the fused (segment-sum + L*src) of a tile of
    K = W*S rows is a matmul with a constant block-diagonal mask [K, W]
    whose weights are 1.0 on neighbor rows and L on the src row:
        psum = segsum + L*src
        out  = relu((0.5/L) * psum)  ==  relu(0.5*src + 0.5*segsum/L)
  - fp32 features are split host-side into bf16 hi + lo halves (exact to
    ~2^-17 relative); hi and lo accumulate into the same PSUM region, so the
    matmul runs at bf16 speed.  One matmul covers 4 subtiles (N=512) via a
    strided rhs access pattern; a 2-bank PSUM tile covers 8 subtiles.
  - Epilogue is a single relu-with-scale per 8 subtiles, read directly from
    PSUM, alternating between ScalarE and VectorE.
  - Rows for each 16-subtile DMA group are packed partition-major host-side
    so every data DMA moves long contiguous per-partition chunks.
  - Host: unpermute the per-core outputs back to node order.

All cores share one SPMD NEFF: bucket sizes are padded to identical values
across cores, so the instruction stream is core-independent; only the DRAM
contents differ.
"""

import math
import os
import sys

import numpy as np

try:
    import concourse.bass as bass  # noqa: F401
except ImportError:
    sys.path.insert(0, "/opt/trn_rl_repo")

import ml_dtypes

import concourse.mybir as mybir
import concourse.tile as tile
from concourse import bacc, bass_utils

BF16 = ml_dtypes.bfloat16
N_CORES = 8
D = 128
G_SUB = int(os.environ.get("K_GSUB", "16"))  # subtiles per data DMA group
B_SUB = 4  # subtiles per matmul (one PSUM bank)
P_SUB = 8  # subtiles per PSUM tile (2 banks)

LAST_EXEC_TIME_NS = None
_CACHE = {}


def _make_plan(seg_len):
    """Bucket structure shared by all cores. seg_len: np.int64 [N]."""
    plan = []
    lengths = np.unique(seg_len)
    counts = {int(L): int((seg_len == L).sum()) for L in lengths}
    for L in sorted(counts):
        cnt = counts[L]
        if cnt == 0:
            continue
        S = L + 1  # rows per segment incl. src row
        assert 2 <= S <= 128, f"segment span {S} unsupported"
        W = 128 // S
        K = W * S
        C = -(-cnt // N_CORES)  # max nodes per core
        Cp = -(-C // W) * W  # pad to whole subtiles
        n_t = Cp // W
        plan.append(dict(L=L, S=S, W=W, K=K, cnt=cnt, Cp=Cp, n_t=n_t))
    return plan


def _iter_groups(plan):
    """Yield (bucket, g0, Gg, rowbase, xoff) in device order.

    rowbase: starting row in the packed data array (p-major within group).
    xoff: starting element in the out flat array ([W, Gg*D] per group).
    """
    rowbase = 0
    xoff = 0
    for b in plan:
        for g0 in range(0, b["n_t"], G_SUB):
            Gg = min(G_SUB, b["n_t"] - g0)
            yield b, g0, Gg, rowbase, xoff
            rowbase += b["K"] * Gg
            xoff += b["W"] * Gg * D


def _plan_sizes(plan):
    R = 0
    X = 0
    for b, g0, Gg, rowbase, xoff in _iter_groups(plan):
        R = rowbase + b["K"] * Gg
        X = xoff + b["W"] * Gg * D
    sumW = sum(b["W"] for b in plan)
    return R, X, sumW


def _build_masks(plan, sumW):
    masks = np.zeros((128, sumW), dtype=BF16)
    moff = 0
    for b in plan:
        L, S, W, K = b["L"], b["S"], b["W"], b["K"]
        k = np.arange(K)
        vals = np.where(k % S < L, 1.0, float(L)).astype(BF16)
        masks[k, moff + k // S] = vals
        moff += W
    return masks


def _build_nc(plan):
    R, X, sumW = _plan_sizes(plan)
    nc = bacc.Bacc("TRN2", target_bir_lowering=False, debug=False,
                   num_devices=N_CORES)
    data = nc.dram_tensor("data", [R, 2 * D], mybir.dt.bfloat16,
                          kind="ExternalInput").ap()
    masks = nc.dram_tensor("masks", [128, sumW], mybir.dt.bfloat16,
                           kind="ExternalInput").ap()
    out = nc.dram_tensor("out", [X], mybir.dt.float32,
                         kind="ExternalOutput").ap()

    Relu = mybir.ActivationFunctionType.Relu
    Alu = mybir.AluOpType

    with tile.TileContext(nc) as tc:
        with tc.tile_pool(name="const", bufs=1) as cpool, \
             tc.tile_pool(name="data", bufs=4) as dpool, \
             tc.tile_pool(name="ps", bufs=3, space="PSUM") as ppool, \
             tc.tile_pool(name="outp", bufs=3) as opool:
            mtile = cpool.tile([128, sumW], mybir.dt.bfloat16)
            nc.sync.dma_start(out=mtile[:, :], in_=masks[:, :])

            moffs = {}
            moff = 0
            for b in plan:
                moffs[b["L"]] = moff
                moff += b["W"]

            gi = 0
            ei = 0
            for b, g0, Gg, rowbase, xoff in _iter_groups(plan):
                L, W, K = b["L"], b["W"], b["K"]
                lhsT = mtile[:K, moffs[L]:moffs[L] + W]
                FG = Gg * D
                scale = float(0.5 / L)

                # Rotate the partition strip for psum/out so partition-poor
                # transfers/ops spread across resources.  Offset must satisfy
                # matmul col tile_position alignment.
                if W <= 32:
                    o = 32 * (gi % 4)
                elif W <= 64:
                    o = 64 * (gi % 2)
                else:
                    o = 0
                gi += 1

                dtile = dpool.tile([128, G_SUB * 2 * D],
                                   mybir.dt.bfloat16, tag="data")
                dsl = data[rowbase:rowbase + K * Gg, :]
                nc.sync.dma_start(
                    out=dtile[:K, :Gg * 2 * D],
                    in_=dsl.rearrange("(p j) c -> p (j c)", p=K))

                otile = opool.tile([128, G_SUB * D], mybir.dt.float32,
                                   tag="outp")

                for p0 in range(0, Gg, P_SUB):
                    Pb = min(P_SUB, Gg - p0)
                    ptile = ppool.tile([128, P_SUB * D],
                                       mybir.dt.float32, tag="ps")
                    for h0 in range(0, Pb, B_SUB):
                        Bb = min(B_SUB, Pb - h0)
                        F = Bb * D
                        r3 = dtile[
                            :K,
                            (p0 + h0) * 2 * D:(p0 + h0 + Bb) * 2 * D
                        ].rearrange("p (j c) -> p j c", c=2 * D)
                        po = ptile[o:o + W, h0 * D:h0 * D + F]
                        nc.tensor.matmul(po, lhsT, r3[:, :, 0:D],
                                         start=True, stop=False,
                                         tile_position=(0, o))
                        nc.tensor.matmul(po, lhsT, r3[:, :, D:2 * D],
                                         start=False, stop=True,
                                         tile_position=(0, o))
                    # relu((0.5/L) * psum), alternating ACT / DVE
                    dst = otile[o:o + W, p0 * D:p0 * D + Pb * D]
                    src_ap = ptile[o:o + W, :Pb * D]
                    if ei % 2 == 0:
                        nc.scalar.activation(dst, src_ap, Relu, scale=scale)
                    else:
                        nc.vector.tensor_scalar(
                            dst, src_ap, scale, 0.0, Alu.mult, Alu.max)
                    ei += 1

                nc.gpsimd.dma_start(
                    out=out[xoff:xoff + W * FG].rearrange(
                        "(p f) -> p f", p=W),
                    in_=otile[o:o + W, :FG])
    nc.compile()
    return nc


def _core_nodes(plan, seg_len, core):
    """Padded node list (−1 = padding) per bucket for one core."""
    per_bucket = []
    for b in plan:
        nodes = np.flatnonzero(seg_len == b["L"])[core::N_CORES]
        pad = np.full(b["Cp"], -1, dtype=np.int64)
        pad[:len(nodes)] = nodes
        per_bucket.append(pad)
    return per_bucket


def _pack_core(plan, node_lists, starts, nbr, src):
    R, X, _ = _plan_sizes(plan)
    T = nbr.shape[0]
    row_idx = np.empty(R, dtype=np.int64)

    bi = {id(b): i for i, b in enumerate(plan)}
    rows_cache = {}
    for b, g0, Gg, rowbase, xoff in _iter_groups(plan):
        L, S, W, K, n_t = b["L"], b["S"], b["W"], b["K"], b["n_t"]
        key = id(b)
        if key not in rows_cache:
            nodes = node_lists[bi[key]]
            nodes0 = np.maximum(nodes, 0)
            st = np.where(nodes >= 0, starts[nodes0], 0)
            seg_rows = np.empty((b["Cp"], S), dtype=np.int64)
            seg_rows[:, :L] = st[:, None] + np.arange(L)[None, :]
            seg_rows[:, L] = T + nodes0  # src row (virtual concat index)
            rows_cache[key] = seg_rows.reshape(n_t, K)
        blk = rows_cache[key][g0:g0 + Gg]  # [Gg, K]
        row_idx[rowbase:rowbase + K * Gg] = blk.T.reshape(-1)

    data = np.empty((R, D), dtype=np.float32)
    m = row_idx < T
    data[m] = nbr[row_idx[m]]
    data[~m] = src[row_idx[~m] - T]
    hi = data.astype(BF16)
    lo = (data - hi.astype(np.float32)).astype(BF16)
    return np.concatenate([hi, lo], axis=1)  # [R, 256] bf16


def _unpack_core(plan, node_lists, out_flat, out_full):
    bi = {id(b): i for i, b in enumerate(plan)}
    vals_cache = {}
    for b, g0, Gg, rowbase, xoff in _iter_groups(plan):
        W = b["W"]
        key = id(b)
        if key not in vals_cache:
            vals_cache[key] = np.empty((b["Cp"], D), dtype=np.float32)
        blk = out_flat[xoff:xoff + W * Gg * D].reshape(W, Gg, D)
        vals_cache[key][g0 * W:(g0 + Gg) * W] = \
            blk.transpose(1, 0, 2).reshape(Gg * W, D)
    for b, nodes in zip(plan, node_lists):
        vals = vals_cache[id(b)]
        real = nodes >= 0
        out_full[nodes[real]] = vals[real]


def kernel(action, src_node_features, neighbor_node_features):
    global LAST_EXEC_TIME_NS
    action = np.asarray(action).astype(np.int64)
    src = np.asarray(src_node_features, dtype=np.float32)
    nbr = np.asarray(neighbor_node_features, dtype=np.float32)
    N = action.shape[0]

    seg_len = action + 1
    starts = np.zeros(N, dtype=np.int64)
    np.cumsum(seg_len[:-1], out=starts[1:])

    plan = _make_plan(seg_len)
    key = tuple((b["L"], b["Cp"]) for b in plan)
    if key in _CACHE:
        nc = _CACHE[key]
    else:
        nc = _build_nc(plan)
        _CACHE[key] = nc

    _, _, sumW = _plan_sizes(plan)
    masks = _build_masks(plan, sumW)

    in_maps = []
    node_lists_all = []
    for c in range(N_CORES):
        node_lists = _core_nodes(plan, seg_len, c)
        node_lists_all.append(node_lists)
        data_flat = _pack_core(plan, node_lists, starts, nbr, src)
        in_maps.append({"data": data_flat, "masks": masks})

    trace = bool(int(os.environ.get("KERNEL_TRACE", "0")))
    res = bass_utils.run_bass_kernel_spmd(
        nc, in_maps, core_ids=list(range(N_CORES)), trace=trace)
    LAST_EXEC_TIME_NS = res.exec_time_ns

    out_full = np.empty((N, D), dtype=np.float32)
    for c in range(N_CORES):
        _unpack_core(plan, node_lists_all[c], res.results[c]["out"],
                     out_full)
    return out_full
